# revision 10
# baseline (speedup 1.0000x reference)
import numpy as np
import ml_dtypes

B, N, DIMG, C, L = 256, 100, 2048, 512, 6
NCORES = 8
BN_EPS = 1e-5
BF16 = ml_dtypes.bfloat16

TRACE = False
LAST_EXEC_NS = None

_NC_CACHE = {}


def _build(S_, L_):
    import concourse.bass as bass  # noqa: F401
    import concourse.tile as tile
    from concourse import bacc, mybir

    f32 = mybir.dt.float32
    b16 = mybir.dt.bfloat16
    AF = mybir.ActivationFunctionType

    assert S_ % 8 == 0
    T_ = S_ * N          # tokens per core
    H = S_ // 2          # samples per half
    HT = H * N           # tokens per half
    LC = L_ * C

    nc = bacc.Bacc("TRN2", target_bir_lowering=False)

    imgT_d = nc.dram_tensor("imgT", [DIMG, T_], b16, kind="ExternalInput").ap()
    wenc_d = nc.dram_tensor("wenc", [DIMG, C], b16, kind="ExternalInput").ap()
    benc_d = nc.dram_tensor("benc", [128, 4], f32, kind="ExternalInput").ap()
    twT_d = nc.dram_tensor("twT", [LC, C], b16, kind="ExternalInput").ap()
    pwT_d = nc.dram_tensor("pwT", [LC, C], b16, kind="ExternalInput").ap()
    gwT_d = nc.dram_tensor("gwT", [LC, C], b16, kind="ExternalInput").ap()
    wwT_d = nc.dram_tensor("wwT", [LC, C], b16, kind="ExternalInput").ap()
    tb2_d = nc.dram_tensor("tb2", [128, 4 * L_], f32, kind="ExternalInput").ap()
    pb2_d = nc.dram_tensor("pb2", [128, 4 * L_], f32, kind="ExternalInput").ap()
    cum2_d = nc.dram_tensor("cum2", [128, 4], f32, kind="ExternalInput").ap()
    gbr_d = nc.dram_tensor("gbr", [1, LC], f32, kind="ExternalInput").ap()
    out_d = nc.dram_tensor("out", [C, T_], f32, kind="ExternalOutput").ap()

    with tile.TileContext(nc) as tc:
        with tc.tile_pool(name="state", bufs=1) as state, \
             tc.tile_pool(name="ps", bufs=6, space="PSUM") as psp:

            xT = [state.tile([128, T_], f32, name=f"xT{c}") for c in range(4)]
            xbu = [state.tile([128, T_], b16, name=f"xbu{c}") for c in range(4)]

            benc_sb = state.tile([128, 4], f32, name="benc")
            tb_sb = state.tile([128, 4 * L_], f32, name="tb")
            pb_sb = state.tile([128, 4 * L_], f32, name="pb")
            cum_sb = state.tile([128, 4], f32, name="cum")
            gbr_sb = state.tile([1, LC], f32, name="gbr")
            ones_f = state.tile([1, 128], f32, name="ones_f")
            nc.vector.memset(ones_f[:, :], 1.0)

            with tc.tile_pool(name="weights", bufs=2) as wp:

                # ---------- encoder: xT = wenc.T @ imgT (+benc) ----------
                # DMA issue order is tuned so PE can start ASAP:
                # (we_k, sl0_k) pairs first, small biases after, layer-0
                # weights trickled in between later slabs.
                with tc.tile_pool(name="enc", bufs=1) as encp, \
                     tc.tile_pool(name="slab", bufs=2) as slabp:
                    wenc_sb = [encp.tile([128, C], b16, name=f"we{k}") for k in range(16)]
                    F0 = 128
                    sl0 = [slabp.tile([128, 512], b16, name=f"sl{k}") for k in range(16)]
                    for k in range(16):
                        nc.sync.dma_start(wenc_sb[k], wenc_d[k * 128:(k + 1) * 128, :])
                        nc.sync.dma_start(sl0[k][:, :F0],
                                          imgT_d[k * 128:(k + 1) * 128, 0:F0])
                        if k == 0:
                            nc.sync.dma_start(benc_sb, benc_d)
                    nc.sync.dma_start(tb_sb, tb2_d)
                    nc.sync.dma_start(pb_sb, pb2_d)
                    nc.sync.dma_start(cum_sb, cum2_d)
                    nc.sync.dma_start(gbr_sb, gbr_d)

                    # slab0: k-outer over 4 psum banks so matmuls start
                    # as soon as each (we_k, sl0_k) pair lands
                    pss = [psp.tile([128, 512], f32, name="psmm") for _ in range(4)]
                    for k in range(16):
                        for cc in range(4):
                            nc.tensor.matmul(pss[cc][:, :F0],
                                             wenc_sb[k][:, cc * 128:(cc + 1) * 128],
                                             sl0[k][:, :F0],
                                             start=(k == 0), stop=(k == 15))
                    for cc in range(4):
                        nc.scalar.activation(xT[cc][:, 0:F0], pss[cc][:, :F0],
                                             AF.Identity, bias=benc_sb[:, cc:cc + 1])
                        nc.scalar.copy(xbu[cc][:, 0:F0], xT[cc][:, 0:F0])

                    # layer-0 weight prefetch tiles (wp ring slot 0)
                    tw0 = [wp.tile([128, C], b16, name=f"tw{k}") for k in range(4)]
                    pw0 = [wp.tile([128, C], b16, name=f"pw{k}") for k in range(4)]
                    gw0 = [wp.tile([128, C], b16, name=f"gw{k}") for k in range(4)]
                    ww0 = [wp.tile([128, C], b16, name=f"ww{k}") for k in range(4)]
                    wgroups = [(tw0, twT_d), (pw0, pwT_d), (gw0, gwT_d), (ww0, wwT_d)]
                    wnext = 0

                    toff = F0
                    while toff < T_:
                        F = min(512, T_ - toff)
                        sl = [slabp.tile([128, 512], b16, name=f"sl{k}") for k in range(16)]
                        for k in range(16):
                            nc.sync.dma_start(sl[k][:, :F],
                                              imgT_d[k * 128:(k + 1) * 128, toff:toff + F])
                        if wnext < 4:
                            tiles, dram = wgroups[wnext]
                            wnext += 1
                            for k in range(4):
                                nc.sync.dma_start(tiles[k], dram[k * 128:(k + 1) * 128, :])
                        for cc in range(4):
                            ps = psp.tile([128, 512], f32, name="psmm")
                            for k in range(16):
                                nc.tensor.matmul(ps[:, :F],
                                                 wenc_sb[k][:, cc * 128:(cc + 1) * 128],
                                                 sl[k][:, :F],
                                                 start=(k == 0), stop=(k == 15))
                            nc.scalar.activation(xT[cc][:, toff:toff + F], ps[:, :F],
                                                 AF.Identity, bias=benc_sb[:, cc:cc + 1])
                            nc.scalar.copy(xbu[cc][:, toff:toff + F],
                                           xT[cc][:, toff:toff + F])
                        toff += F
                    while wnext < 4:
                        tiles, dram = wgroups[wnext]
                        wnext += 1
                        for k in range(4):
                            nc.sync.dma_start(tiles[k], dram[k * 128:(k + 1) * 128, :])

                # ---------- layers ----------
                with tc.tile_pool(name="lp", bufs=1) as lp, \
                     tc.tile_pool(name="rt", bufs=2) as rtp, \
                     tc.tile_pool(name="wtmp", bufs=3) as wtp:
                  for l in range(L_):
                    if l == 0:
                        tw_sb, pw_sb, gw_sb, ww_sb = tw0, pw0, gw0, ww0
                    else:
                        tw_sb = [wp.tile([128, C], b16, name=f"tw{k}") for k in range(4)]
                        pw_sb = [wp.tile([128, C], b16, name=f"pw{k}") for k in range(4)]
                        gw_sb = [wp.tile([128, C], b16, name=f"gw{k}") for k in range(4)]
                        ww_sb = [wp.tile([128, C], b16, name=f"ww{k}") for k in range(4)]
                        for k in range(4):
                            r0 = (l * 4 + k) * 128
                            nc.sync.dma_start(tw_sb[k], twT_d[r0:r0 + 128, :])
                            nc.sync.dma_start(pw_sb[k], pwT_d[r0:r0 + 128, :])
                            nc.sync.dma_start(gw_sb[k], gwT_d[r0:r0 + 128, :])
                            nc.sync.dma_start(ww_sb[k], wwT_d[r0:r0 + 128, :])

                    # broadcast g bias to all partitions (once per layer)
                    gbB = lp.tile([128, C], f32, name="gbB")
                    ps = psp.tile([128, 512], f32, name="psmm")
                    nc.tensor.matmul(ps, ones_f, gbr_sb[0:1, l * C:(l + 1) * C],
                                     start=True, stop=True)
                    nc.scalar.copy(gbB, ps)

                    for h in range(2):
                        hoff = h * HT
                        ngrp = H // 4

                        # th / ph projections (unpadded tokens)
                        thT = [lp.tile([128, HT], b16, name=f"thT{c}") for c in range(4)]
                        phT = [lp.tile([128, HT], b16, name=f"phT{c}") for c in range(4)]
                        for grp in range(ngrp):
                            co = hoff + grp * 400
                            for ic in range(4):
                                ps = psp.tile([128, 512], f32, name="psmm")
                                for k in range(4):
                                    nc.tensor.matmul(ps[:, :400],
                                                     tw_sb[k][:, ic * 128:(ic + 1) * 128],
                                                     xbu[k][:, co:co + 400],
                                                     start=(k == 0), stop=(k == 3))
                                nc.scalar.activation(thT[ic][:, grp * 400:(grp + 1) * 400],
                                                     ps[:, :400], AF.Identity,
                                                     bias=tb_sb[:, l * 4 + ic:l * 4 + ic + 1])
                            for ic in range(4):
                                ps = psp.tile([128, 512], f32, name="psmm")
                                for k in range(4):
                                    nc.tensor.matmul(ps[:, :400],
                                                     pw_sb[k][:, ic * 128:(ic + 1) * 128],
                                                     xbu[k][:, co:co + 400],
                                                     start=(k == 0), stop=(k == 3))
                                nc.scalar.activation(phT[ic][:, grp * 400:(grp + 1) * 400],
                                                     ps[:, :400], AF.Identity,
                                                     bias=pb_sb[:, l * 4 + ic:l * 4 + ic + 1])

                        # g: natural layout [token, chan] per sample, + broadcast bias
                        gna = lp.tile([128, H * C], b16, name="gna")
                        for s in range(H):
                            so = hoff + s * N
                            ps = psp.tile([128, 512], f32, name="psmm")
                            for k in range(4):
                                nc.tensor.matmul(ps[0:N, :], xbu[k][:, so:so + N],
                                                 gw_sb[k], start=(k == 0), stop=(k == 3))
                            nc.vector.tensor_add(gna[0:N, s * C:(s + 1) * C],
                                                 ps[0:N, :], gbB[0:N, :])

                        # RT + y + wy, software-pipelined per 4-sample group:
                        # RT runs one group ahead, wy one group behind.
                        yT = [lp.tile([128, HT], b16, name=f"yT{c}") for c in range(4)]

                        def do_rt(grp):
                            psr = psp.tile([128, 512], f32, name="psrt", bufs=2)
                            for s4 in range(4):
                                s = grp * 4 + s4
                                for ic in range(4):
                                    nc.tensor.matmul(psr[0:N, s4 * N:(s4 + 1) * N],
                                                     phT[ic][:, s * N:(s + 1) * N],
                                                     thT[ic][:, s * N:(s + 1) * N],
                                                     start=(ic == 0), stop=(ic == 3))
                            rt4 = rtp.tile([128, 400], b16, name="rt4")
                            nc.scalar.copy(rt4[0:N, :], psr[0:N, 0:400])
                            return rt4

                        def do_y(grp, rt4):
                            for jc in range(4):
                                ps = psp.tile([128, 512], f32, name="psmm")
                                for s4 in range(4):
                                    s = grp * 4 + s4
                                    nc.tensor.matmul(
                                        ps[:, s4 * N:(s4 + 1) * N],
                                        gna[0:N, s * C + jc * 128:s * C + (jc + 1) * 128],
                                        rt4[0:N, s4 * N:(s4 + 1) * N],
                                        start=True, stop=True)
                                dst = yT[jc][:, grp * 400:(grp + 1) * 400]
                                if jc % 2 == 0:
                                    nc.vector.tensor_copy(dst, ps[:, :400])
                                else:
                                    nc.scalar.copy(dst, ps[:, :400])

                        def do_wy(grp):
                            for oc in range(4):
                                ps = psp.tile([128, 512], f32, name="psmm")
                                for jc in range(4):
                                    nc.tensor.matmul(ps[:, :400],
                                                     ww_sb[jc][:, oc * 128:(oc + 1) * 128],
                                                     yT[jc][:, grp * 400:(grp + 1) * 400],
                                                     start=(jc == 0), stop=(jc == 3))
                                xv = xT[oc][:, hoff + grp * 400:hoff + (grp + 1) * 400]
                                if l < L_ - 1:
                                    nc.vector.tensor_add(xv, xv, ps[:, :400])
                                    nc.gpsimd.tensor_copy(
                                        xbu[oc][:, hoff + grp * 400:hoff + (grp + 1) * 400],
                                        xv)
                                else:
                                    wt = wtp.tile([128, 400], f32, name="wtmp")
                                    nc.scalar.activation(wt, ps[:, :400], AF.Identity,
                                                         bias=cum_sb[:, oc:oc + 1])
                                    nc.vector.tensor_add(xv, xv, wt)
                                    nc.sync.dma_start(
                                        out_d[oc * 128:(oc + 1) * 128,
                                              hoff + grp * 400:hoff + (grp + 1) * 400],
                                        xv)

                        rt4s = {}
                        for step in range(ngrp + 2):
                            if step < ngrp:
                                rt4s[step] = do_rt(step)
                            if 1 <= step <= ngrp:
                                do_y(step - 1, rt4s.pop(step - 1))
                            if step >= 2:
                                do_wy(step - 2)

    nc.compile()
    return nc


def _get_nc(S_, L_):
    key = (S_, L_)
    if key not in _NC_CACHE:
        _NC_CACHE[key] = _build(S_, L_)
    return _NC_CACHE[key]


def _prep_weights(trans_w, trans_b, gw, gb, tw, tb, pw, pb, ww, wb,
                  bn_gamma, bn_beta, bn_mean, bn_var, L_):
    inv = bn_gamma / np.sqrt(bn_var + BN_EPS)
    biasl = (wb - bn_mean) * inv + bn_beta          # [L,512] additive BN constant
    cumprev = np.cumsum(biasl, axis=0) - biasl      # sum of biasl[j] for j < l
    cum_last = biasl.sum(axis=0)
    tb_f = (tb + np.einsum("lij,lj->li", tw, cumprev)) / N
    pb_f = pb + np.einsum("lij,lj->li", pw, cumprev)
    gb_f = gb + np.einsum("lij,lj->li", gw, cumprev)
    LC = L_ * C
    d = {
        "wenc": np.ascontiguousarray(trans_w.T).astype(BF16),
        "benc": np.ascontiguousarray(trans_b.reshape(4, 128).T),
        "twT": np.ascontiguousarray((tw.transpose(0, 2, 1) / N).reshape(LC, C)).astype(BF16),
        "pwT": np.ascontiguousarray(pw.transpose(0, 2, 1).reshape(LC, C)).astype(BF16),
        "gwT": np.ascontiguousarray(gw.transpose(0, 2, 1).reshape(LC, C)).astype(BF16),
        "wwT": np.ascontiguousarray((ww.transpose(0, 2, 1) * inv[:, None, :]).reshape(LC, C)).astype(BF16),
        "tb2": np.ascontiguousarray(tb_f.reshape(L_, 4, 128).transpose(2, 0, 1).reshape(128, 4 * L_)),
        "pb2": np.ascontiguousarray(pb_f.reshape(L_, 4, 128).transpose(2, 0, 1).reshape(128, 4 * L_)),
        "cum2": np.ascontiguousarray(cum_last.reshape(4, 128).T),
        "gbr": np.ascontiguousarray(gb_f.reshape(1, LC)),
    }
    for k in d:
        if d[k].dtype == np.float64:
            d[k] = d[k].astype(np.float32)
    return d


def _run(img, weights, S_, L_):
    global LAST_EXEC_NS
    from concourse.bass_utils import run_bass_kernel_spmd

    nb = img.shape[0]
    ncores = nb // S_
    assert nb == ncores * S_
    wmap = _prep_weights(L_=L_, **weights)
    nc = _get_nc(S_, L_)
    in_maps = []
    for i in range(ncores):
        sl = np.ascontiguousarray(
            img[i * S_:(i + 1) * S_].reshape(S_ * N, DIMG).T).astype(BF16)
        m = {"imgT": sl}
        m.update(wmap)
        in_maps.append(m)

    kwargs = {}
    if TRACE:
        _register_ntff_hook()
        kwargs["trace"] = True
    res = run_bass_kernel_spmd(nc, in_maps, core_ids=list(range(ncores)), **kwargs)
    LAST_EXEC_NS = res.exec_time_ns

    full = np.empty((nb, N, C), np.float32)
    for i in range(ncores):
        full[i * S_:(i + 1) * S_] = res.results[i]["out"].T.reshape(S_, N, C)
    return full


def _register_ntff_hook():
    import sys, types
    if "antenv.axon_hooks" in sys.modules:
        return
    sys.path.insert(0, "/root/.axon_site/trn_agent_boot")
    import trn_boot
    hook = trn_boot._ntff_profile_via_ctypes("/opt/axon/libaxon_pjrt.so")
    mod = types.ModuleType("antenv.axon_hooks")
    mod.get_axon_ntff_profile_hook = lambda: hook
    mod.set_axon_ntff_profile_hook = lambda h: None
    sys.modules["antenv.axon_hooks"] = mod


def kernel(**inputs):
    img = np.asarray(inputs["img"], np.float32)
    weights = {k: np.asarray(v, np.float32) for k, v in inputs.items() if k != "img"}
    return _run(img, weights, B // NCORES, L)


# revision 16
# speedup vs baseline: 1.0109x; 1.0109x over previous
import numpy as np
import ml_dtypes

B, N, DIMG, C, L = 256, 100, 2048, 512, 6
NCORES = 8
BN_EPS = 1e-5
BF16 = ml_dtypes.bfloat16

TRACE = False
LAST_EXEC_NS = None

_NC_CACHE = {}


def _build(S_, L_):
    import concourse.bass as bass  # noqa: F401
    import concourse.tile as tile
    from concourse import bacc, mybir

    f32 = mybir.dt.float32
    b16 = mybir.dt.bfloat16
    AF = mybir.ActivationFunctionType

    assert S_ % 8 == 0
    T_ = S_ * N          # tokens per core
    H = S_ // 2          # samples per half
    HT = H * N           # tokens per half
    LC = L_ * C
    F0 = T_ % 512 or 512          # first (ragged) slab size
    NSLAB = (T_ - F0) // 512      # remaining full 512-token slabs

    nc = bacc.Bacc("TRN2", target_bir_lowering=False)

    # img pre-tiled host-side into block-contiguous slabs for full DMA BW
    imgA_d = nc.dram_tensor("imgA", [DIMG, F0], b16, kind="ExternalInput").ap()
    imgB_d = nc.dram_tensor("imgB", [NSLAB * DIMG, 512], b16, kind="ExternalInput").ap()
    wenc_d = nc.dram_tensor("wenc", [DIMG, C], b16, kind="ExternalInput").ap()
    benc_d = nc.dram_tensor("benc", [128, 4], f32, kind="ExternalInput").ap()
    twT_d = nc.dram_tensor("twT", [LC, C], b16, kind="ExternalInput").ap()
    pwT_d = nc.dram_tensor("pwT", [LC, C], b16, kind="ExternalInput").ap()
    gwT_d = nc.dram_tensor("gwT", [LC, C], b16, kind="ExternalInput").ap()
    wwT_d = nc.dram_tensor("wwT", [LC, C], b16, kind="ExternalInput").ap()
    tb2_d = nc.dram_tensor("tb2", [128, 4 * L_], f32, kind="ExternalInput").ap()
    pb2_d = nc.dram_tensor("pb2", [128, 4 * L_], f32, kind="ExternalInput").ap()
    cum2_d = nc.dram_tensor("cum2", [128, 4], f32, kind="ExternalInput").ap()
    gbr_d = nc.dram_tensor("gbr", [1, LC], f32, kind="ExternalInput").ap()
    # output as contiguous [h, grp, oc, 128, 400] blocks (host reassembles)
    out_d = nc.dram_tensor("out", [S_ * 128, 400], f32, kind="ExternalOutput").ap()

    with tile.TileContext(nc) as tc:
        with tc.tile_pool(name="state", bufs=1) as state, \
             tc.tile_pool(name="ps", bufs=6, space="PSUM") as psp:

            xT = [state.tile([128, T_], f32, name=f"xT{c}") for c in range(4)]
            xbu = [state.tile([128, T_], b16, name=f"xbu{c}") for c in range(4)]

            benc_sb = state.tile([128, 4], f32, name="benc")
            tb_sb = state.tile([128, 4 * L_], f32, name="tb")
            pb_sb = state.tile([128, 4 * L_], f32, name="pb")
            cum_sb = state.tile([128, 4], f32, name="cum")
            gbr_sb = state.tile([1, LC], f32, name="gbr")
            ones_f = state.tile([1, 128], f32, name="ones_f")
            nc.vector.memset(ones_f[:, :], 1.0)

            with tc.tile_pool(name="weights", bufs=2) as wp:

                # ---------- encoder: xT = wenc.T @ imgT (+benc) ----------
                # DMA issue order is tuned so PE can start ASAP:
                # (we_k, sl0_k) pairs first, small biases after, layer-0
                # weights trickled in between later slabs.
                with tc.tile_pool(name="enc", bufs=1) as encp, \
                     tc.tile_pool(name="slab", bufs=3) as slabp:
                    wenc_sb = [encp.tile([128, C], b16, name=f"we{k}") for k in range(16)]
                    sl0 = [slabp.tile([128, 512], b16, name=f"sl{k}") for k in range(16)]
                    for k in range(16):
                        nc.sync.dma_start(wenc_sb[k], wenc_d[k * 128:(k + 1) * 128, :])
                        nc.sync.dma_start(sl0[k][:, :F0],
                                          imgA_d[k * 128:(k + 1) * 128, :])
                        if k == 0:
                            nc.sync.dma_start(benc_sb, benc_d)
                    nc.sync.dma_start(tb_sb, tb2_d)
                    nc.sync.dma_start(pb_sb, pb2_d)
                    nc.sync.dma_start(cum_sb, cum2_d)
                    nc.sync.dma_start(gbr_sb, gbr_d)

                    # slab0: k-outer over 4 psum banks so matmuls start
                    # as soon as each (we_k, sl0_k) pair lands
                    pss = [psp.tile([128, 512], f32, name="psmm") for _ in range(4)]
                    for k in range(16):
                        for cc in range(4):
                            nc.tensor.matmul(pss[cc][:, :F0],
                                             wenc_sb[k][:, cc * 128:(cc + 1) * 128],
                                             sl0[k][:, :F0],
                                             start=(k == 0), stop=(k == 15))
                    for cc in range(4):
                        nc.scalar.activation(xT[cc][:, 0:F0], pss[cc][:, :F0],
                                             AF.Identity, bias=benc_sb[:, cc:cc + 1])
                        nc.scalar.copy(xbu[cc][:, 0:F0], xT[cc][:, 0:F0])

                    # layer-0 weight prefetch tiles (wp ring slot 0)
                    tw0 = [wp.tile([128, C], b16, name=f"tw{k}") for k in range(4)]
                    pw0 = [wp.tile([128, C], b16, name=f"pw{k}") for k in range(4)]
                    gw0 = [wp.tile([128, C], b16, name=f"gw{k}") for k in range(4)]
                    ww0 = [wp.tile([128, C], b16, name=f"ww{k}") for k in range(4)]
                    wgroups = [(tw0, twT_d), (pw0, pwT_d), (gw0, gwT_d), (ww0, wwT_d)]
                    wnext = 0

                    toff = F0
                    while toff < T_:
                        F = 512
                        s_i = (toff - F0) // 512
                        sl = [slabp.tile([128, 512], b16, name=f"sl{k}") for k in range(16)]
                        for k in range(16):
                            blk = s_i * 16 + k
                            nc.sync.dma_start(sl[k][:, :F],
                                              imgB_d[blk * 128:(blk + 1) * 128, :])
                        if wnext < 4:
                            tiles, dram = wgroups[wnext]
                            wnext += 1
                            for k in range(4):
                                nc.sync.dma_start(tiles[k], dram[k * 128:(k + 1) * 128, :])
                        for cc in range(4):
                            ps = psp.tile([128, 512], f32, name="psmm")
                            for k in range(16):
                                nc.tensor.matmul(ps[:, :F],
                                                 wenc_sb[k][:, cc * 128:(cc + 1) * 128],
                                                 sl[k][:, :F],
                                                 start=(k == 0), stop=(k == 15))
                            nc.scalar.activation(xT[cc][:, toff:toff + F], ps[:, :F],
                                                 AF.Identity, bias=benc_sb[:, cc:cc + 1])
                            nc.scalar.copy(xbu[cc][:, toff:toff + F],
                                           xT[cc][:, toff:toff + F])
                        toff += F
                    while wnext < 4:
                        tiles, dram = wgroups[wnext]
                        wnext += 1
                        for k in range(4):
                            nc.sync.dma_start(tiles[k], dram[k * 128:(k + 1) * 128, :])

                # ---------- layers ----------
                with tc.tile_pool(name="lp", bufs=1) as lp, \
                     tc.tile_pool(name="rt", bufs=2) as rtp, \
                     tc.tile_pool(name="wtmp", bufs=3) as wtp:
                  for l in range(L_):
                    if l == 0:
                        tw_sb, pw_sb, gw_sb, ww_sb = tw0, pw0, gw0, ww0
                    else:
                        tw_sb = [wp.tile([128, C], b16, name=f"tw{k}") for k in range(4)]
                        pw_sb = [wp.tile([128, C], b16, name=f"pw{k}") for k in range(4)]
                        gw_sb = [wp.tile([128, C], b16, name=f"gw{k}") for k in range(4)]
                        ww_sb = [wp.tile([128, C], b16, name=f"ww{k}") for k in range(4)]
                        for k in range(4):
                            r0 = (l * 4 + k) * 128
                            nc.sync.dma_start(tw_sb[k], twT_d[r0:r0 + 128, :])
                            nc.sync.dma_start(pw_sb[k], pwT_d[r0:r0 + 128, :])
                            nc.sync.dma_start(gw_sb[k], gwT_d[r0:r0 + 128, :])
                            nc.sync.dma_start(ww_sb[k], wwT_d[r0:r0 + 128, :])

                    # broadcast g bias to all partitions (once per layer)
                    gbB = lp.tile([128, C], f32, name="gbB")
                    ps = psp.tile([128, 512], f32, name="psmm")
                    nc.tensor.matmul(ps, ones_f, gbr_sb[0:1, l * C:(l + 1) * C],
                                     start=True, stop=True)
                    nc.scalar.copy(gbB, ps)

                    for h in range(2):
                        hoff = h * HT
                        ngrp = H // 4

                        # th / ph projections (unpadded tokens)
                        thT = [lp.tile([128, HT], b16, name=f"thT{c}") for c in range(4)]
                        phT = [lp.tile([128, HT], b16, name=f"phT{c}") for c in range(4)]
                        for grp in range(ngrp):
                            co = hoff + grp * 400
                            for ic in range(4):
                                ps = psp.tile([128, 512], f32, name="psmm")
                                for k in range(4):
                                    nc.tensor.matmul(ps[:, :400],
                                                     tw_sb[k][:, ic * 128:(ic + 1) * 128],
                                                     xbu[k][:, co:co + 400],
                                                     start=(k == 0), stop=(k == 3))
                                nc.scalar.activation(thT[ic][:, grp * 400:(grp + 1) * 400],
                                                     ps[:, :400], AF.Identity,
                                                     bias=tb_sb[:, l * 4 + ic:l * 4 + ic + 1])
                            for ic in range(4):
                                ps = psp.tile([128, 512], f32, name="psmm")
                                for k in range(4):
                                    nc.tensor.matmul(ps[:, :400],
                                                     pw_sb[k][:, ic * 128:(ic + 1) * 128],
                                                     xbu[k][:, co:co + 400],
                                                     start=(k == 0), stop=(k == 3))
                                nc.scalar.activation(phT[ic][:, grp * 400:(grp + 1) * 400],
                                                     ps[:, :400], AF.Identity,
                                                     bias=pb_sb[:, l * 4 + ic:l * 4 + ic + 1])

                        # g: natural layout [token, chan] per sample, + broadcast bias
                        gna = lp.tile([128, H * C], b16, name="gna")
                        for s in range(H):
                            so = hoff + s * N
                            ps = psp.tile([128, 512], f32, name="psmm")
                            for k in range(4):
                                nc.tensor.matmul(ps[0:N, :], xbu[k][:, so:so + N],
                                                 gw_sb[k], start=(k == 0), stop=(k == 3))
                            nc.vector.tensor_add(gna[0:N, s * C:(s + 1) * C],
                                                 ps[0:N, :], gbB[0:N, :])

                        # RT + y + wy, software-pipelined per 4-sample group:
                        # RT runs one group ahead, wy one group behind.
                        yT = [lp.tile([128, HT], b16, name=f"yT{c}") for c in range(4)]

                        def do_rt(grp):
                            psr = psp.tile([128, 512], f32, name="psrt", bufs=2)
                            for s4 in range(4):
                                s = grp * 4 + s4
                                for ic in range(4):
                                    nc.tensor.matmul(psr[0:N, s4 * N:(s4 + 1) * N],
                                                     phT[ic][:, s * N:(s + 1) * N],
                                                     thT[ic][:, s * N:(s + 1) * N],
                                                     start=(ic == 0), stop=(ic == 3))
                            rt4 = rtp.tile([128, 400], b16, name="rt4")
                            nc.scalar.copy(rt4[0:N, :], psr[0:N, 0:400])
                            return rt4

                        def do_y(grp, rt4):
                            for jc in range(4):
                                ps = psp.tile([128, 512], f32, name="psmm")
                                for s4 in range(4):
                                    s = grp * 4 + s4
                                    nc.tensor.matmul(
                                        ps[:, s4 * N:(s4 + 1) * N],
                                        gna[0:N, s * C + jc * 128:s * C + (jc + 1) * 128],
                                        rt4[0:N, s4 * N:(s4 + 1) * N],
                                        start=True, stop=True)
                                dst = yT[jc][:, grp * 400:(grp + 1) * 400]
                                if jc % 2 == 0:
                                    nc.vector.tensor_copy(dst, ps[:, :400])
                                else:
                                    nc.scalar.copy(dst, ps[:, :400])

                        def do_wy(grp):
                            for oc in range(4):
                                ps = psp.tile([128, 512], f32, name="psmm")
                                for jc in range(4):
                                    nc.tensor.matmul(ps[:, :400],
                                                     ww_sb[jc][:, oc * 128:(oc + 1) * 128],
                                                     yT[jc][:, grp * 400:(grp + 1) * 400],
                                                     start=(jc == 0), stop=(jc == 3))
                                xv = xT[oc][:, hoff + grp * 400:hoff + (grp + 1) * 400]
                                if l < L_ - 1:
                                    nc.vector.tensor_add(xv, xv, ps[:, :400])
                                    nc.gpsimd.tensor_copy(
                                        xbu[oc][:, hoff + grp * 400:hoff + (grp + 1) * 400],
                                        xv)
                                else:
                                    wt = wtp.tile([128, 400], f32, name="wtmp")
                                    nc.scalar.activation(wt, ps[:, :400], AF.Identity,
                                                         bias=cum_sb[:, oc:oc + 1])
                                    nc.vector.tensor_add(xv, xv, wt)
                                    b = (h * ngrp + grp) * 4 + oc
                                    nc.sync.dma_start(
                                        out_d[b * 128:(b + 1) * 128, :], xv)

                        rt4s = {}
                        for step in range(ngrp + 2):
                            if step < ngrp:
                                rt4s[step] = do_rt(step)
                            if 1 <= step <= ngrp:
                                do_y(step - 1, rt4s.pop(step - 1))
                            if step >= 2:
                                do_wy(step - 2)

    nc.compile()
    return nc


def _get_nc(S_, L_):
    key = (S_, L_)
    if key not in _NC_CACHE:
        _NC_CACHE[key] = _build(S_, L_)
    return _NC_CACHE[key]


def _prep_weights(trans_w, trans_b, gw, gb, tw, tb, pw, pb, ww, wb,
                  bn_gamma, bn_beta, bn_mean, bn_var, L_):
    inv = bn_gamma / np.sqrt(bn_var + BN_EPS)
    biasl = (wb - bn_mean) * inv + bn_beta          # [L,512] additive BN constant
    cumprev = np.cumsum(biasl, axis=0) - biasl      # sum of biasl[j] for j < l
    cum_last = biasl.sum(axis=0)
    tb_f = (tb + np.einsum("lij,lj->li", tw, cumprev)) / N
    pb_f = pb + np.einsum("lij,lj->li", pw, cumprev)
    gb_f = gb + np.einsum("lij,lj->li", gw, cumprev)
    LC = L_ * C
    d = {
        "wenc": np.ascontiguousarray(trans_w.T).astype(BF16),
        "benc": np.ascontiguousarray(trans_b.reshape(4, 128).T),
        "twT": np.ascontiguousarray((tw.transpose(0, 2, 1) / N).reshape(LC, C)).astype(BF16),
        "pwT": np.ascontiguousarray(pw.transpose(0, 2, 1).reshape(LC, C)).astype(BF16),
        "gwT": np.ascontiguousarray(gw.transpose(0, 2, 1).reshape(LC, C)).astype(BF16),
        "wwT": np.ascontiguousarray((ww.transpose(0, 2, 1) * inv[:, None, :]).reshape(LC, C)).astype(BF16),
        "tb2": np.ascontiguousarray(tb_f.reshape(L_, 4, 128).transpose(2, 0, 1).reshape(128, 4 * L_)),
        "pb2": np.ascontiguousarray(pb_f.reshape(L_, 4, 128).transpose(2, 0, 1).reshape(128, 4 * L_)),
        "cum2": np.ascontiguousarray(cum_last.reshape(4, 128).T),
        "gbr": np.ascontiguousarray(gb_f.reshape(1, LC)),
    }
    for k in d:
        if d[k].dtype == np.float64:
            d[k] = d[k].astype(np.float32)
    return d


def _run(img, weights, S_, L_):
    global LAST_EXEC_NS
    from concourse.bass_utils import run_bass_kernel_spmd

    nb = img.shape[0]
    ncores = nb // S_
    assert nb == ncores * S_
    T_ = S_ * N
    F0 = T_ % 512 or 512
    nslab = (T_ - F0) // 512
    ngrp = S_ // 8
    wmap = _prep_weights(L_=L_, **weights)
    nc = _get_nc(S_, L_)
    in_maps = []
    for i in range(ncores):
        sl = img[i * S_:(i + 1) * S_].reshape(T_, DIMG).T.astype(BF16)
        imgA = np.ascontiguousarray(sl[:, :F0])
        imgB = np.ascontiguousarray(
            sl[:, F0:].reshape(16, 128, nslab, 512)
            .transpose(2, 0, 1, 3).reshape(nslab * DIMG, 512))
        m = {"imgA": imgA, "imgB": imgB}
        m.update(wmap)
        in_maps.append(m)

    kwargs = {}
    if TRACE:
        _register_ntff_hook()
        kwargs["trace"] = True
    res = run_bass_kernel_spmd(nc, in_maps, core_ids=list(range(ncores)), **kwargs)
    LAST_EXEC_NS = res.exec_time_ns

    full = np.empty((nb, N, C), np.float32)
    for i in range(ncores):
        r = res.results[i]["out"].reshape(2, ngrp, 4, 128, 400)
        outT = r.transpose(2, 3, 0, 1, 4).reshape(C, T_)
        full[i * S_:(i + 1) * S_] = outT.T.reshape(S_, N, C)
    return full


def _register_ntff_hook():
    import sys, types
    if "antenv.axon_hooks" in sys.modules:
        return
    sys.path.insert(0, "/root/.axon_site/trn_agent_boot")
    import trn_boot
    hook = trn_boot._ntff_profile_via_ctypes("/opt/axon/libaxon_pjrt.so")
    mod = types.ModuleType("antenv.axon_hooks")
    mod.get_axon_ntff_profile_hook = lambda: hook
    mod.set_axon_ntff_profile_hook = lambda h: None
    sys.modules["antenv.axon_hooks"] = mod


def kernel(**inputs):
    img = np.asarray(inputs["img"], np.float32)
    weights = {k: np.asarray(v, np.float32) for k, v in inputs.items() if k != "img"}
    return _run(img, weights, B // NCORES, L)
